# revision 1
# baseline (speedup 1.0000x reference)
"""Multi-scale deformable attention — TRN2 Bass kernel.

Sharding: data-parallel over batch (bs=8 -> one batch element per NeuronCore).
Host (numpy) computes the small control-plane tensors (sampling locations,
bilinear+attention weights, gather/weighted-sum of projected values); each
core runs the output projection (900x256 @ 256x256 matmul over 2 K-tiles,
fp32 PE) fused with bias + residual add, via bass_utils.run_bass_kernel_spmd
on cores 0-7. Output is re-assembled to the full (nq, bs, C) array.
"""
import sys

for _p in ("/opt/trn_rl_repo", "/opt/trn_rl_repo/concourse"):
    if _p not in sys.path:
        sys.path.insert(0, _p)

import numpy as np
from contextlib import ExitStack

import concourse.bass as bass
import concourse.tile as tile
from concourse import bacc, mybir
from concourse.bass_utils import run_bass_kernel_spmd

F32 = mybir.dt.float32

# Static problem config (matches reference.py / spec.json)
SPATIAL = [(128, 128), (64, 64), (32, 32), (16, 16)]
NH, NL, NP, C = 8, 4, 4, 256
HD = C // NH  # 32
NQ, BS = 900, 8
NQP = 1024  # padded queries
N_CORES = 8

_COMPILED = {}


def _build_nc():
    """Out-proj + residual kernel: out = preT.T @ w + qres, per core."""
    nc = bacc.Bacc("TRN2", target_bir_lowering=False, debug=False)
    preT = nc.dram_tensor("preT", [C, NQP], F32, kind="ExternalInput").ap()
    w = nc.dram_tensor("w", [C, C], F32, kind="ExternalInput").ap()
    qres = nc.dram_tensor("qres", [NQP, C], F32, kind="ExternalInput").ap()
    out = nc.dram_tensor("out", [NQP, C], F32, kind="ExternalOutput").ap()

    with tile.TileContext(nc) as tc, ExitStack() as ctx:
        lpool = ctx.enter_context(tc.tile_pool(name="lhs", bufs=3))
        rpool = ctx.enter_context(tc.tile_pool(name="rhs", bufs=1))
        qpool = ctx.enter_context(tc.tile_pool(name="qres", bufs=3))
        opool = ctx.enter_context(tc.tile_pool(name="out", bufs=3))
        ppool = ctx.enter_context(tc.tile_pool(name="ps", bufs=3, space="PSUM"))

        wts = []
        for k in range(2):
            wk = rpool.tile([128, C], F32, tag=f"w{k}")
            nc.sync.dma_start(wk[:], w[k * 128:(k + 1) * 128, :])
            wts.append(wk)

        for t in range(NQP // 128):
            lts = []
            for k in range(2):
                lk = lpool.tile([128, 128], F32, tag=f"l{k}")
                nc.sync.dma_start(lk[:], preT[k * 128:(k + 1) * 128,
                                              t * 128:(t + 1) * 128])
                lts.append(lk)
            qt = qpool.tile([128, C], F32)
            nc.sync.dma_start(qt[:], qres[t * 128:(t + 1) * 128, :])

            ps = ppool.tile([128, C], F32)
            for k in range(2):
                nc.tensor.matmul(
                    ps[:],
                    lts[k][:],
                    wts[k][:],
                    start=(k == 0),
                    stop=(k == 1),
                )
            ot = opool.tile([128, C], F32)
            nc.vector.tensor_tensor(ot[:], ps[:], qt[:], mybir.AluOpType.add)
            nc.sync.dma_start(out[t * 128:(t + 1) * 128, :], ot[:])

    nc.compile()
    return nc


def _build_nc_val():
    """Value projection: val[r, n] = sum_k vT[k, r] * W_valT[k, n], per core."""
    NV = 21760
    nc = bacc.Bacc("TRN2", target_bir_lowering=False, debug=False)
    vT = nc.dram_tensor("vT", [C, NV], F32, kind="ExternalInput").ap()
    w = nc.dram_tensor("w", [C, C], F32, kind="ExternalInput").ap()
    val = nc.dram_tensor("val", [NV, C], F32, kind="ExternalOutput").ap()
    F32R = mybir.dt.float32r

    with tile.TileContext(nc) as tc, ExitStack() as ctx:
        lpool = ctx.enter_context(tc.tile_pool(name="lhs", bufs=4))
        rpool = ctx.enter_context(tc.tile_pool(name="rhs", bufs=1))
        opool = ctx.enter_context(tc.tile_pool(name="out", bufs=4))
        ppool = ctx.enter_context(tc.tile_pool(name="ps", bufs=4, space="PSUM"))

        wts = []
        for k in range(2):
            wk = rpool.tile([128, C], F32, tag=f"w{k}")
            nc.sync.dma_start(wk[:], w[k * 128:(k + 1) * 128, :])
            wts.append(wk)

        for t in range(NV // 128):
            lts = []
            for k in range(2):
                lk = lpool.tile([128, 128], F32, tag=f"l{k}")
                nc.sync.dma_start(lk[:], vT[k * 128:(k + 1) * 128,
                                            t * 128:(t + 1) * 128])
                lts.append(lk)
            ps = ppool.tile([128, C], F32)
            for k in range(2):
                nc.tensor.matmul(
                    ps[:],
                    lts[k][:],
                    wts[k][:],
                    start=(k == 0),
                    stop=(k == 1),
                )
            ot = opool.tile([128, C], F32)
            nc.scalar.copy(ot[:], ps[:])
            nc.sync.dma_start(val[t * 128:(t + 1) * 128, :], ot[:])

    nc.compile()
    return nc


def _host_pre(query, value, reference_points, W_off, b_off, W_attn, b_attn,
              W_val, b_val, val_dev=None):
    """Everything up to (but excluding) the output projection, in numpy fp32.

    val_dev: optional (bs, nv, C) device-computed value projection (pre-bias).
    Returns pre: (bs, nq, C) == the einsum output of the reference.
    """
    q = np.transpose(query, (1, 0, 2)).astype(np.float32)   # (bs, nq, C)
    v = np.transpose(value, (1, 0, 2)).astype(np.float32)   # (bs, nv, C)
    bs, nq, _ = q.shape
    nv = v.shape[1]

    if val_dev is not None:
        val = val_dev + b_val
    else:
        val = v @ W_val.T + b_val                            # (bs, nv, C)
    val = val.reshape(bs, nv, NH, HD).transpose(0, 2, 1, 3)  # (bs, nh, nv, hd)

    off = (q @ W_off.T + b_off).reshape(bs, nq, NH, NL, NP, 2)
    logits = (q @ W_attn.T + b_attn).reshape(bs, nq, NH, NL * NP)
    logits = logits - logits.max(axis=-1, keepdims=True)
    e = np.exp(logits)
    attn = (e / e.sum(axis=-1, keepdims=True)).reshape(bs, nq, NH, NL, NP)

    norm = np.array([[w_, h_] for h_, w_ in SPATIAL], np.float32)  # (NL, 2)
    loc = reference_points[:, :, None, :, None, :] + off / norm[None, None, None, :, None, :]

    pre = np.zeros((bs, nq, NH, HD), np.float32)
    start = 0
    for l, (H, W) in enumerate(SPATIAL):
        vl = val[:, :, start:start + H * W, :]     # (bs, nh, H*W, hd)
        lc = loc[:, :, :, l]                       # (bs, nq, nh, np, 2)
        x = lc[..., 0] * W - 0.5
        y = lc[..., 1] * H - 0.5
        x0 = np.floor(x)
        y0 = np.floor(y)
        tx = (x - x0).astype(np.float32)
        ty = (y - y0).astype(np.float32)
        x0i = x0.astype(np.int64)
        y0i = y0.astype(np.int64)
        a_l = attn[:, :, :, l]                     # (bs, nq, nh, np)? -> (bs,nq,NH,NP)
        for dy, wy in ((0, 1.0 - ty), (1, ty)):
            for dx, wx in ((0, 1.0 - tx), (1, tx)):
                xi = x0i + dx
                yi = y0i + dy
                valid = ((xi >= 0) & (xi < W) & (yi >= 0) & (yi < H)).astype(np.float32)
                idx = np.clip(yi, 0, H - 1) * W + np.clip(xi, 0, W - 1)  # (bs,nq,nh,np)
                wgt = (wx * wy * valid).astype(np.float32) * a_l         # (bs,nq,nh,np)
                # g[b,qq,h,p,:] = vl[b,h,idx[b,qq,h,p],:]
                bi = np.arange(bs)[:, None, None, None]
                hi = np.arange(NH)[None, None, :, None]
                g = vl[bi, hi, idx]                 # (bs, nq, nh, np, hd)
                pre += (wgt[..., None] * g).sum(axis=3)
        start += H * W
    return pre.reshape(bs, nq, C)


def kernel(**inputs):
    query = np.asarray(inputs["query"], np.float32)
    value = np.asarray(inputs["value"], np.float32)
    reference_points = np.asarray(inputs["reference_points"], np.float32)
    W_off = np.asarray(inputs["W_off"], np.float32)
    b_off = np.asarray(inputs["b_off"], np.float32)
    W_attn = np.asarray(inputs["W_attn"], np.float32)
    b_attn = np.asarray(inputs["b_attn"], np.float32)
    W_val = np.asarray(inputs["W_val"], np.float32)
    b_val = np.asarray(inputs["b_val"], np.float32)
    W_out = np.asarray(inputs["W_out"], np.float32)
    b_out = np.asarray(inputs["b_out"], np.float32)

    if "nc" not in _COMPILED:
        _COMPILED["nc"] = _build_nc()
        _COMPILED["nc_val"] = _build_nc_val()
    nc = _COMPILED["nc"]

    # --- device stage 1: value projection, one batch element per core ---
    w_val_rhs = np.ascontiguousarray(W_val.T)
    in_maps_v = []
    for b in range(N_CORES):
        vT = np.ascontiguousarray(value[:, b, :].T)         # (C, nv)
        in_maps_v.append({"vT": vT, "w": w_val_rhs})
    res_v = run_bass_kernel_spmd(_COMPILED["nc_val"], in_maps_v,
                                 core_ids=list(range(N_CORES)))
    val_dev = np.stack([res_v.results[b]["val"] for b in range(N_CORES)], axis=0)

    pre = _host_pre(query, value, reference_points, W_off, b_off,
                    W_attn, b_attn, W_val, b_val, val_dev=val_dev)  # (bs, nq, C)

    w_rhs = np.ascontiguousarray(W_out.T)                   # rhs [k, n]
    in_maps = []
    for b in range(N_CORES):
        preT = np.zeros((C, NQP), np.float32)
        preT[:, :NQ] = pre[b].T                             # lhsT [k, m=q]
        qres = np.zeros((NQP, C), np.float32)
        qres[:NQ] = query[:, b, :] + b_out[None, :]         # residual + bias
        in_maps.append({"preT": preT, "w": w_rhs, "qres": qres})

    res = run_bass_kernel_spmd(nc, in_maps, core_ids=list(range(N_CORES)))
    outs = [res.results[b]["out"][:NQ] for b in range(N_CORES)]  # (nq, C) each
    full = np.stack(outs, axis=1).astype(np.float32)        # (nq, bs, C)
    return full



# revision 21
# speedup vs baseline: 199.6283x; 199.6283x over previous
"""Multi-scale deformable attention — TRN2 Bass kernel, fully on-device.

Sharding: data-parallel over batch (bs=8 -> one batch element per NeuronCore).
Each core runs the ENTIRE computation for its batch element in one NEFF:
  1. value projection (v @ W_val.T + b_val) -> val scratch in HBM
     (v transposed on-device via PE identity transposes)
  2. query-side: off/attn projections (PE), softmax (ACT+DVE), sampling
     locations, bilinear weights and gather row indices (DVE)
  3. 2048 per-partition indirect-DMA gathers (gpsimd SWDGE): each pulls a
     288-element contiguous span val[row, h*32 : ...] covering both
     x-corners (row, row+1) of one (head, level-point, y-corner, q-tile)
  4. weighted sum over (level-point, y-corner, x-corner) on DVE -> pre
  5. output projection + bias + residual (PE) -> out
Host only pads/slices per-core arrays and re-assembles the full output.

The x-corner pair is folded into one gather: row = loff + yc*W + xs with
xs = clip(x0, 0, W-2); pair weights
  wp0 = (1-tx)*[0<=x0<=W-2] + tx*[x0==-1]
  wp1 = tx*[0<=x0<=W-2] + (1-tx)*[x0==W-1]
reproduce the reference's per-corner valid masks exactly.
"""
import sys

for _p in ("/opt/trn_rl_repo", "/opt/trn_rl_repo/concourse"):
    if _p not in sys.path:
        sys.path.insert(0, _p)

import numpy as np
from contextlib import ExitStack

import concourse.bass as bass
import concourse.tile as tile
from concourse import bacc, mybir
from concourse.bass import IndirectOffsetOnAxis
from concourse.bass_utils import run_bass_kernel_spmd
from concourse.masks import make_identity

F32 = mybir.dt.float32
I32 = mybir.dt.int32
AF = mybir.ActivationFunctionType
OP = mybir.AluOpType
AX = mybir.AxisListType

# Static problem config (matches reference.py / spec.json)
SPATIAL = [(128, 128), (64, 64), (32, 32), (16, 16)]
LOFF = [0, 16384, 20480, 21504]
NH, NL, NPT, C, HD = 8, 4, 4, 256, 32
NQ, QP, BS, NV = 900, 1024, 8, 21760
P = 128
NQT = QP // P          # 8 query tiles
NCH = NV // P          # 170 value chunks
SPAN = C + HD          # 288: gathered span covers (row, h*32) .. (row+1, h*32+32)
N_CORES = 8

_COMPILED = {}


def _bc(ap, sizes):
    """Append stride-0 broadcast dims to an AP."""
    return ap.to_broadcast(list(ap.shape) + [int(s) for s in sizes])


def _build_nc():
    nc = bacc.Bacc("TRN2", target_bir_lowering=False, debug=False)
    q = nc.dram_tensor("q", [QP, C], F32, kind="ExternalInput").ap()
    v = nc.dram_tensor("v", [NV, C], F32, kind="ExternalInput").ap()
    ref = nc.dram_tensor("ref", [QP, NL * 2], F32, kind="ExternalInput").ap()
    wofft = nc.dram_tensor("wofft", [C, C], F32, kind="ExternalInput").ap()
    boff = nc.dram_tensor("boff", [1, C], F32, kind="ExternalInput").ap()
    wattnt = nc.dram_tensor("wattnt", [C, 128], F32, kind="ExternalInput").ap()
    battn = nc.dram_tensor("battn", [1, 128], F32, kind="ExternalInput").ap()
    wvalt = nc.dram_tensor("wvalt", [C, C], F32, kind="ExternalInput").ap()
    bval = nc.dram_tensor("bval", [1, C], F32, kind="ExternalInput").ap()
    woutt = nc.dram_tensor("woutt", [C, C], F32, kind="ExternalInput").ap()
    bout = nc.dram_tensor("bout", [1, C], F32, kind="ExternalInput").ap()
    out = nc.dram_tensor("out", [QP, C], F32, kind="ExternalOutput").ap()
    # span reads end exactly at row NV-1 (xs <= W-2 keeps row+1 in-level)
    val = nc.dram_tensor("valbuf", [NV, C], F32, kind="Internal").ap()

    with tile.TileContext(nc) as tc, ExitStack() as ctx:
        pp = ctx.enter_context(tc.tile_pool(name="pers", bufs=1))
        wk = ctx.enter_context(tc.tile_pool(name="wk", bufs=1))
        gpool = ctx.enter_context(tc.tile_pool(name="g", bufs=2))
        vin_p = ctx.enter_context(tc.tile_pool(name="vin", bufs=3))
        vt_p = ctx.enter_context(tc.tile_pool(name="vt", bufs=3))
        vout_p = ctx.enter_context(tc.tile_pool(name="vout", bufs=3))
        ps_tp = ctx.enter_context(tc.tile_pool(name="ps_tp", bufs=2, space="PSUM"))
        ps_mm = ctx.enter_context(tc.tile_pool(name="ps_mm", bufs=2, space="PSUM"))
        ps_at = ctx.enter_context(tc.tile_pool(name="ps_at", bufs=2, space="PSUM"))

        # ---- constants & weights ----
        ident = pp.tile([P, P], F32)
        make_identity(nc, ident)
        ones1 = pp.tile([1, P], F32)
        nc.gpsimd.memset(ones1, 1.0)

        wofft_sb = pp.tile([P, 2, C], F32)
        wattnt_sb = pp.tile([P, 2, 128], F32)
        wvalt_sb = pp.tile([P, 2, C], F32)
        woutt_sb = pp.tile([P, 2, C], F32)
        for k in range(2):
            nc.sync.dma_start(wofft_sb[:, k], wofft[k * P:(k + 1) * P, :])
            nc.sync.dma_start(wattnt_sb[:, k], wattnt[k * P:(k + 1) * P, :])
            nc.sync.dma_start(wvalt_sb[:, k], wvalt[k * P:(k + 1) * P, :])
            nc.sync.dma_start(woutt_sb[:, k], woutt[k * P:(k + 1) * P, :])
        boff_sb = pp.tile([1, C], F32)
        battn_sb = pp.tile([1, 128], F32)
        bval_sb = pp.tile([1, C], F32)
        bout_sb = pp.tile([1, C], F32)
        nc.sync.dma_start(boff_sb[:], boff[:])
        nc.sync.dma_start(battn_sb[:], battn[:])
        nc.sync.dma_start(bval_sb[:], bval[:])
        nc.sync.dma_start(bout_sb[:], bout[:])

        # ---- load q tiles + reference points ----
        qsb = pp.tile([P, NQT, C], F32)
        for qt in range(NQT):
            nc.sync.dma_start(qsb[:, qt], q[qt * P:(qt + 1) * P, :])
        ref_sb = wk.tile([P, NQT, NL, 2], F32, tag="refs")
        nc.sync.dma_start(
            ref_sb.rearrange("p qt l x -> p qt (l x)"),
            ref.rearrange("(qt p) c -> p qt c", p=P)
        )

        # ---- qT via PE transpose, then off/attn projections ----
        qT = pp.tile([P, NQT, C], F32)
        off_sb = wk.tile([P, NQT, C], F32, tag="s8a")
        ssum = wk.tile([P, NQT, NH], F32, tag="ssum")
        rinv = wk.tile([P, NQT, NH], F32, tag="rinv")
        attn_sb = wk.tile([P, NQT, NH, 16], F32, tag="attns")
        for qt in range(NQT):
            pst = ps_tp.tile([P, C], F32, tag="tp")
            for k in range(2):
                nc.tensor.transpose(pst[:, k * P:(k + 1) * P],
                                    qsb[:, qt, k * P:(k + 1) * P], ident[:])
            nc.scalar.copy(qT[:, qt], pst[:])

            psm = ps_mm.tile([P, C], F32, tag="mm")
            nc.tensor.matmul(psm[:], qT[:, qt, 0:P], wofft_sb[:, 0],
                             start=True, stop=False)
            nc.tensor.matmul(psm[:], qT[:, qt, P:C], wofft_sb[:, 1],
                             start=False, stop=False)
            nc.tensor.matmul(psm[:], ones1[:], boff_sb[:],
                             start=False, stop=True)
            nc.scalar.copy(off_sb[:, qt], psm[:])

            psa = ps_at.tile([P, 128], F32, tag="at")
            nc.tensor.matmul(psa[:], qT[:, qt, 0:P], wattnt_sb[:, 0],
                             start=True, stop=False)
            nc.tensor.matmul(psa[:], qT[:, qt, P:C], wattnt_sb[:, 1],
                             start=False, stop=False)
            nc.tensor.matmul(psa[:], ones1[:], battn_sb[:],
                             start=False, stop=True)
            # softmax over the 16 (l,pt) slots per head (no max-sub: |logit|<~3)
            nc.scalar.activation(
                attn_sb[:, qt].rearrange("p h l -> p (h l)"), psa[:], AF.Exp)
            nc.vector.tensor_reduce(
                ssum[:, qt], attn_sb[:, qt], axis=AX.X, op=OP.add)
            nc.vector.reciprocal(rinv[:, qt], ssum[:, qt])
            nc.vector.tensor_tensor(
                attn_sb[:, qt], attn_sb[:, qt],
                _bc(rinv[:, qt], [16]),
                OP.mult)

        # ---- sampling coordinates x,y  (x = ref_x*W + off_x - 0.5) ----
        X = wk.tile([P, 1024], F32, tag="X")     # becomes tx in place
        Y = wk.tile([P, 1024], F32, tag="Y")     # becomes ty in place
        X0 = wk.tile([P, 1024], F32, tag="X0")
        Y0 = wk.tile([P, 1024], F32, tag="Y0")
        U = wk.tile([P, 1024], F32, tag="U")
        SC = wk.tile([P, 1024], F32, tag="SC")
        XI = wk.tile([P, 1024], I32, tag="XI")
        refw = wk.tile([P, NQT, 2], F32, tag="refw")

        def v4(t):  # [128,1024] -> [128, qt, h, l, pt]
            return t.rearrange("p (qt h l pt) -> p qt h l pt",
                               qt=NQT, h=NH, l=NL, pt=NPT)

        off_v = off_sb.rearrange("p qt (h l pt xy) -> p qt h l pt xy",
                                 h=NH, l=NL, pt=NPT, xy=2)
        for l, (Hl, Wl) in enumerate(SPATIAL):
            nc.vector.tensor_scalar(refw[:, :, 0], ref_sb[:, :, l, 0],
                                    float(Wl), -0.5, op0=OP.mult, op1=OP.add)
            nc.vector.tensor_scalar(refw[:, :, 1], ref_sb[:, :, l, 1],
                                    float(Hl), -0.5, op0=OP.mult, op1=OP.add)
            nc.vector.tensor_tensor(v4(X)[:, :, :, l, :],
                                    off_v[:, :, :, l, :, 0],
                                    _bc(refw[:, :, 0], [NH, NPT]), OP.add)
            nc.vector.tensor_tensor(v4(Y)[:, :, :, l, :],
                                    off_v[:, :, :, l, :, 1],
                                    _bc(refw[:, :, 1], [NH, NPT]), OP.add)

        # ---- floor -> X0/Y0 (exact for |x| < 2^23), frac -> X/Y in place ----
        for (A, A0) in ((X, X0), (Y, Y0)):
            nc.vector.tensor_copy(XI[:], A[:])          # f32 -> i32 (round)
            nc.vector.tensor_copy(A0[:], XI[:])         # i32 -> f32
            nc.vector.tensor_tensor(U[:], A0[:], A[:], OP.is_gt)
            nc.vector.tensor_tensor(A0[:], A0[:], U[:], OP.subtract)
            nc.vector.tensor_tensor(A[:], A[:], A0[:], OP.subtract)  # frac

        # ---- x-side: xs = clip(x0,0,W-2), pair weights wp0/wp1 ----
        XS = wk.tile([P, 1024], F32, tag="XI")  # reuse i32 floor scratch slot
        MA = wk.tile([P, 1024], F32, tag="MA")
        MB = wk.tile([P, 1024], F32, tag="MB")
        MC = wk.tile([P, 1024], F32, tag="MC")
        WPX = wk.tile([P, NQT, NH, 16, 2], F32, tag="WPX")
        for l, (Hl, Wl) in enumerate(SPATIAL):
            x0l = v4(X0)[:, :, :, l, :]
            nc.vector.tensor_scalar(v4(XS)[:, :, :, l, :], x0l,
                                    float(Wl - 2), 0.0, op0=OP.min, op1=OP.max)
            nc.vector.tensor_scalar(v4(SC)[:, :, :, l, :], x0l,
                                    0.0, None, op0=OP.is_ge)
            nc.vector.tensor_scalar(v4(MA)[:, :, :, l, :], x0l,
                                    float(Wl - 2), None, op0=OP.is_le)
            nc.vector.tensor_tensor(v4(MA)[:, :, :, l, :],
                                    v4(MA)[:, :, :, l, :],
                                    v4(SC)[:, :, :, l, :], OP.mult)
            nc.vector.tensor_scalar(v4(MB)[:, :, :, l, :], x0l,
                                    -1.0, None, op0=OP.is_equal)
            nc.vector.tensor_scalar(v4(MC)[:, :, :, l, :], x0l,
                                    float(Wl - 1), None, op0=OP.is_equal)
        wpx_v = WPX.rearrange("p qt h l x -> p (qt h l) x")
        nc.vector.tensor_scalar(U[:], X[:], -1.0, 1.0, op0=OP.mult, op1=OP.add)
        nc.vector.tensor_tensor(SC[:], X[:], MB[:], OP.mult)
        nc.vector.tensor_tensor(wpx_v[:, :, 0], U[:], MA[:], OP.mult)
        nc.vector.tensor_tensor(wpx_v[:, :, 0], wpx_v[:, :, 0], SC[:], OP.add)
        nc.vector.tensor_tensor(SC[:], U[:], MC[:], OP.mult)
        nc.vector.tensor_tensor(wpx_v[:, :, 1], X[:], MA[:], OP.mult)
        nc.vector.tensor_tensor(wpx_v[:, :, 1], wpx_v[:, :, 1], SC[:], OP.add)

        # ---- y-side: wy(dy)*my(dy)*attn ----
        WY = wk.tile([P, NQT, NH, 16, 2], F32, tag="WY")
        attn_f = attn_sb.rearrange("p qt h l -> p (qt h l)")
        wy_v = WY.rearrange("p qt h l y -> p (qt h l) y")
        nc.vector.tensor_scalar(U[:], Y[:], -1.0, 1.0, op0=OP.mult, op1=OP.add)
        for dy in range(2):
            for l, (Hl, Wl) in enumerate(SPATIAL):
                y0l = v4(Y0)[:, :, :, l, :]
                nc.vector.tensor_scalar(v4(SC)[:, :, :, l, :], y0l,
                                        float(-dy), None, op0=OP.is_ge)
                nc.vector.tensor_scalar(v4(MA)[:, :, :, l, :], y0l,
                                        float(Hl - 1 - dy), None, op0=OP.is_le)
                nc.vector.tensor_tensor(v4(MA)[:, :, :, l, :],
                                        v4(MA)[:, :, :, l, :],
                                        v4(SC)[:, :, :, l, :], OP.mult)
            nc.vector.tensor_tensor(wy_v[:, :, dy], MA[:],
                                    Y[:] if dy else U[:], OP.mult)
            nc.vector.tensor_tensor(wy_v[:, :, dy], wy_v[:, :, dy],
                                    attn_f, OP.mult)

        # ---- combined weights WF[qt, h, y, lp, xj] = WY * WPX ----
        WF = pp.tile([P, NQT, NH, 2, 16, 2], F32)
        for dy in range(2):
            for xj in range(2):
                nc.vector.tensor_tensor(
                    WF[:, :, :, dy, :, xj],
                    WY[:, :, :, :, dy], WPX[:, :, :, :, xj], OP.mult)

        # ---- gather row indices IDX2[qt, h, lp, y] = loff + yc*W + xs ----
        IDX2 = pp.tile([P, NQT, NH, 16, 2], I32)
        IDXF = wk.tile([P, 2048], F32, tag="s8a")  # reuse off_sb slot
        idxf_v = IDXF.rearrange("p (qt h l pt y) -> p qt h l pt y",
                                qt=NQT, h=NH, l=NL, pt=NPT, y=2)
        for l, (Hl, Wl) in enumerate(SPATIAL):
            for dy in range(2):
                sl = v4(SC)[:, :, :, l, :]
                nc.vector.tensor_scalar(sl, v4(Y0)[:, :, :, l, :],
                                        float(dy), 0.0, op0=OP.add, op1=OP.max)
                nc.vector.tensor_scalar(sl, sl, float(Hl - 1), None, op0=OP.min)
                nc.vector.tensor_scalar(sl, sl, float(Wl), float(LOFF[l]),
                                        op0=OP.mult, op1=OP.add)
                nc.vector.tensor_tensor(idxf_v[:, :, :, l, :, dy], sl,
                                        v4(XS)[:, :, :, l, :], OP.add)
        nc.vector.tensor_copy(IDX2.rearrange("p qt h l y -> p (qt h l y)"),
                              IDXF[:])

        # ---- value projection: val = v @ W_val.T + b_val ----
        for ch in range(NCH):
            vin = vin_p.tile([P, C], F32, tag="vin")
            nc.sync.dma_start(vin[:], v[ch * P:(ch + 1) * P, :])
            pst = ps_tp.tile([P, C], F32, tag="tp")
            for k in range(2):
                nc.tensor.transpose(pst[:, k * P:(k + 1) * P],
                                    vin[:, k * P:(k + 1) * P], ident[:])
            vt = vt_p.tile([P, C], F32, tag="vt")
            nc.scalar.copy(vt[:], pst[:])
            psv = ps_mm.tile([P, C], F32, tag="mm")
            nc.tensor.matmul(psv[:], vt[:, 0:P], wvalt_sb[:, 0],
                             start=True, stop=False)
            nc.tensor.matmul(psv[:], vt[:, P:C], wvalt_sb[:, 1],
                             start=False, stop=False)
            nc.tensor.matmul(psv[:], ones1[:], bval_sb[:],
                             start=False, stop=True)
            vout = vout_p.tile([P, C], F32, tag="vout")
            nc.scalar.copy(vout[:], psv[:])
            nc.sync.dma_start(val[ch * P:(ch + 1) * P, :], vout[:])

        # ---- gathers + weighted sum ----
        pre = pp.tile([P, NQT, NH, HD], F32)
        TMP = wk.tile([P, 16, 2, HD], F32, tag="X")    # reuse tx slot
        TMP2 = wk.tile([P, HD], F32, tag="Y")          # reuse ty slot
        for h in range(NH):
            for qt in range(NQT):
                for dy in range(2):
                    G = gpool.tile([P, 16, SPAN], F32, tag="G")
                    for lp in range(16):
                        nc.gpsimd.indirect_dma_start(
                            out=G[:, lp],
                            out_offset=None,
                            in_=val,
                            in_offset=IndirectOffsetOnAxis(
                                ap=IDX2[:, qt, h, lp, dy:dy + 1], axis=0),
                            element_offset=h * HD,
                        )
                    # tmp[lp, xj, c] = G[lp, xj*256 + c] * WF[qt,h,dy,lp,xj]
                    for xj in range(2):
                        gsl = bass.AP(G.tensor, G.offset + xj * C,
                                      [list(G.ap[0]), [SPAN, 16], [1, HD]])
                        nc.vector.scalar_tensor_tensor(
                            TMP[:, :, xj, :], gsl, 1.0,
                            _bc(WF[:, qt, h, dy, :, xj], [HD]),
                            op0=OP.mult, op1=OP.mult)
                    red_out = pre[:, qt, h] if dy == 0 else TMP2[:]
                    nc.vector.tensor_reduce(
                        red_out,
                        TMP.rearrange("p l x d -> p d l x"),
                        axis=AX.XY, op=OP.add)
                    if dy == 1:
                        nc.vector.tensor_tensor(pre[:, qt, h], pre[:, qt, h],
                                                TMP2[:], OP.add)

        # ---- output projection + bias + residual ----
        for qt in range(NQT):
            pst = ps_tp.tile([P, C], F32, tag="tp")
            pre_f = pre[:, qt].rearrange("p h d -> p (h d)")
            for k in range(2):
                nc.tensor.transpose(pst[:, k * P:(k + 1) * P],
                                    pre_f[:, k * P:(k + 1) * P], ident[:])
            pT = vt_p.tile([P, C], F32, tag="vt")
            nc.scalar.copy(pT[:], pst[:])
            pso = ps_mm.tile([P, C], F32, tag="mm")
            nc.tensor.matmul(pso[:], pT[:, 0:P], woutt_sb[:, 0],
                             start=True, stop=False)
            nc.tensor.matmul(pso[:], pT[:, P:C], woutt_sb[:, 1],
                             start=False, stop=False)
            nc.tensor.matmul(pso[:], ones1[:], bout_sb[:],
                             start=False, stop=True)
            osb = vout_p.tile([P, C], F32, tag="vout")
            nc.vector.tensor_tensor(osb[:], pso[:], qsb[:, qt], OP.add)
            nc.sync.dma_start(out[qt * P:(qt + 1) * P, :], osb[:])

    nc.compile()
    return nc


def _make_in_maps(query, value, reference_points, W_off, b_off, W_attn,
                  b_attn, W_val, b_val, W_out, b_out):
    wofft = np.ascontiguousarray(W_off.T)
    wattnt = np.ascontiguousarray(W_attn.T)
    wvalt = np.ascontiguousarray(W_val.T)
    woutt = np.ascontiguousarray(W_out.T)
    shared = {
        "wofft": wofft, "boff": b_off.reshape(1, C),
        "wattnt": wattnt, "battn": b_attn.reshape(1, 128),
        "wvalt": wvalt, "bval": b_val.reshape(1, C),
        "woutt": woutt, "bout": b_out.reshape(1, C),
    }
    in_maps = []
    for b in range(N_CORES):
        qp = np.zeros((QP, C), np.float32)
        qp[:NQ] = query[:, b, :]
        refp = np.full((QP, NL * 2), 0.5, np.float32)
        refp[:NQ] = reference_points[b].reshape(NQ, NL * 2)
        in_maps.append({
            "q": qp,
            "v": np.ascontiguousarray(value[:, b, :]),
            "ref": refp,
            **shared,
        })
    return in_maps


def _build_exec(nc):
    """Memoized jitted SPMD callable mirroring bass2jax.run_bass_via_pjrt's
    multi-core path, so repeat calls skip retracing and inputs can be staged
    on device for timing."""
    import jax
    from jax.experimental.shard_map import shard_map
    from jax.sharding import Mesh, PartitionSpec
    from concourse import bass2jax, mybir as mb

    bass2jax.install_neuronx_cc_hook()
    in_names, out_names, out_avals, zero_outs = [], [], [], []
    partition_name = (nc.partition_id_tensor.name
                      if nc.partition_id_tensor else None)
    for alloc in nc.m.functions[0].allocations:
        if not isinstance(alloc, mb.MemoryLocationSet):
            continue
        name = alloc.memorylocations[0].name
        if alloc.kind == "ExternalInput":
            if name != partition_name:
                in_names.append(name)
        elif alloc.kind == "ExternalOutput":
            shape = tuple(alloc.tensor_shape)
            dtype = mb.dt.np(alloc.dtype)
            out_names.append(name)
            out_avals.append(jax.core.ShapedArray(shape, dtype))
            zero_outs.append(np.zeros(shape, dtype))
    n_params = len(in_names)
    all_names = list(in_names) + list(out_names)
    if partition_name is not None:
        all_names.append(partition_name)
    donate = tuple(range(n_params, n_params + len(out_names)))

    def _body(*args):
        operands = list(args)
        if partition_name is not None:
            operands.append(bass2jax.partition_id_tensor())
        return tuple(bass2jax._bass_exec_p.bind(
            *operands,
            out_avals=tuple(out_avals),
            in_names=tuple(all_names),
            out_names=tuple(out_names),
            lowering_input_output_aliases=(),
            sim_require_finite=True,
            sim_require_nnan=True,
            nc=nc,
        ))

    devices = jax.devices()[:N_CORES]
    mesh = Mesh(np.asarray(devices), ("core",))
    in_specs = (PartitionSpec("core"),) * (n_params + len(out_names))
    out_specs = (PartitionSpec("core"),) * len(out_names)
    fn = jax.jit(
        shard_map(_body, mesh=mesh, in_specs=in_specs, out_specs=out_specs,
                  check_rep=False),
        donate_argnums=donate, keep_unused=True)
    return {
        "fn": fn, "mesh": mesh, "in_names": in_names,
        "out_names": out_names, "zero_outs": zero_outs,
        "out_avals": out_avals,
    }


def _exec(in_maps, timeit=0):
    """Run the compiled kernel on 8 cores. Returns (results, best_ns).
    timeit>0: additionally re-run the executable with device-staged inputs
    `timeit` times and report the best wall-clock ns."""
    import jax, time
    from jax.sharding import NamedSharding, PartitionSpec

    if "exec" not in _COMPILED:
        _COMPILED["exec"] = _build_exec(_COMPILED["nc"])
    ex = _COMPILED["exec"]
    fn, mesh = ex["fn"], ex["mesh"]
    concat_in = [
        np.concatenate([np.asarray(m[name]) for m in in_maps], axis=0)
        for name in ex["in_names"]
    ]
    concat_zeros = [
        np.zeros((N_CORES * z.shape[0], *z.shape[1:]), z.dtype)
        for z in ex["zero_outs"]
    ]
    out_arrs = fn(*concat_in, *concat_zeros)
    jax.block_until_ready(out_arrs)
    results = [
        {name: np.asarray(out_arrs[i]).reshape(
            N_CORES, *ex["out_avals"][i].shape)[c]
         for i, name in enumerate(ex["out_names"])}
        for c in range(N_CORES)
    ]
    best_ns = None
    if timeit:
        shard = NamedSharding(mesh, PartitionSpec("core"))
        staged = [jax.device_put(x, shard) for x in concat_in]
        jax.block_until_ready(staged)
        for _ in range(timeit):
            zo = [jax.device_put(z, shard) for z in concat_zeros]
            jax.block_until_ready(zo)
            t0 = time.perf_counter()
            o = fn(*staged, *zo)
            jax.block_until_ready(o)
            dt = time.perf_counter() - t0
            best_ns = dt * 1e9 if best_ns is None else min(best_ns, dt * 1e9)
    return results, best_ns


def kernel(**inputs):
    query = np.asarray(inputs["query"], np.float32)
    value = np.asarray(inputs["value"], np.float32)
    reference_points = np.asarray(inputs["reference_points"], np.float32)
    W_off = np.asarray(inputs["W_off"], np.float32)
    b_off = np.asarray(inputs["b_off"], np.float32)
    W_attn = np.asarray(inputs["W_attn"], np.float32)
    b_attn = np.asarray(inputs["b_attn"], np.float32)
    W_val = np.asarray(inputs["W_val"], np.float32)
    b_val = np.asarray(inputs["b_val"], np.float32)
    W_out = np.asarray(inputs["W_out"], np.float32)
    b_out = np.asarray(inputs["b_out"], np.float32)

    if "nc" not in _COMPILED:
        _COMPILED["nc"] = _build_nc()
    nc = _COMPILED["nc"]

    in_maps = _make_in_maps(query, value, reference_points, W_off, b_off,
                            W_attn, b_attn, W_val, b_val, W_out, b_out)
    results, _ = _exec(in_maps)
    outs = [results[b]["out"][:NQ] for b in range(N_CORES)]
    return np.stack(outs, axis=1).astype(np.float32)


# revision 25
# speedup vs baseline: 4049.7817x; 20.2866x over previous
"""Multi-scale deformable attention — TRN2 Bass kernel, fully on-device.

Sharding: data-parallel over batch (bs=8 -> one batch element per NeuronCore).
Each core runs the ENTIRE computation for its batch element in one NEFF:
  1. value projection (v @ W_val.T + b_val) -> val scratch in HBM
     (v transposed on-device via PE identity transposes)
  2. query-side: off/attn projections (PE), softmax (ACT+DVE), sampling
     locations, bilinear weights and gather row indices (DVE)
  3. 2048 per-partition indirect-DMA gathers (gpsimd SWDGE): each pulls a
     288-element contiguous span val[row, h*32 : ...] covering both
     x-corners (row, row+1) of one (head, level-point, y-corner, q-tile)
  4. weighted sum over (level-point, y-corner, x-corner) on DVE -> pre
  5. output projection + bias + residual (PE) -> out
Host only pads/slices per-core arrays and re-assembles the full output.

The x-corner pair is folded into one gather: row = loff + yc*W + xs with
xs = clip(x0, 0, W-2); pair weights
  wp0 = (1-tx)*[0<=x0<=W-2] + tx*[x0==-1]
  wp1 = tx*[0<=x0<=W-2] + (1-tx)*[x0==W-1]
reproduce the reference's per-corner valid masks exactly.
"""
import sys

for _p in ("/opt/trn_rl_repo", "/opt/trn_rl_repo/concourse"):
    if _p not in sys.path:
        sys.path.insert(0, _p)

import numpy as np
from contextlib import ExitStack

import concourse.bass as bass
import concourse.tile as tile
from concourse import bacc, mybir
from concourse.bass import IndirectOffsetOnAxis
from concourse.bass_utils import run_bass_kernel_spmd
from concourse.masks import make_identity

F32 = mybir.dt.float32
I32 = mybir.dt.int32
AF = mybir.ActivationFunctionType
OP = mybir.AluOpType
AX = mybir.AxisListType

# Static problem config (matches reference.py / spec.json)
SPATIAL = [(128, 128), (64, 64), (32, 32), (16, 16)]
LOFF = [0, 16384, 20480, 21504]
NH, NL, NPT, C, HD = 8, 4, 4, 256, 32
NQ, QP, BS, NV = 900, 1024, 8, 21760
P = 128
NQT = QP // P          # 8 query tiles
NCH = NV // P          # 170 value chunks
SPAN = C + HD          # 288: gathered span covers (row, h*32) .. (row+1, h*32+32)
N_CORES = 8

_COMPILED = {}


def _bc(ap, sizes):
    """Append stride-0 broadcast dims to an AP."""
    return ap.to_broadcast(list(ap.shape) + [int(s) for s in sizes])


def _build_nc():
    nc = bacc.Bacc("TRN2", target_bir_lowering=False, debug=False)
    q = nc.dram_tensor("q", [QP, C], F32, kind="ExternalInput").ap()
    v = nc.dram_tensor("v", [NV, C], F32, kind="ExternalInput").ap()
    ref = nc.dram_tensor("ref", [QP, NL * 2], F32, kind="ExternalInput").ap()
    wofft = nc.dram_tensor("wofft", [C, C], F32, kind="ExternalInput").ap()
    boff = nc.dram_tensor("boff", [1, C], F32, kind="ExternalInput").ap()
    wattnt = nc.dram_tensor("wattnt", [C, 128], F32, kind="ExternalInput").ap()
    battn = nc.dram_tensor("battn", [1, 128], F32, kind="ExternalInput").ap()
    wvalt = nc.dram_tensor("wvalt", [C, C], F32, kind="ExternalInput").ap()
    bval = nc.dram_tensor("bval", [1, C], F32, kind="ExternalInput").ap()
    woutt = nc.dram_tensor("woutt", [C, C], F32, kind="ExternalInput").ap()
    bout = nc.dram_tensor("bout", [1, C], F32, kind="ExternalInput").ap()
    out = nc.dram_tensor("out", [QP, C], F32, kind="ExternalOutput").ap()
    # span reads end exactly at row NV-1 (xs <= W-2 keeps row+1 in-level)
    val = nc.dram_tensor("valbuf", [NV, C], F32, kind="Internal").ap()

    with tile.TileContext(nc) as tc, ExitStack() as ctx:
        pp = ctx.enter_context(tc.tile_pool(name="pers", bufs=1))
        wk = ctx.enter_context(tc.tile_pool(name="wk", bufs=1))
        gpool = ctx.enter_context(tc.tile_pool(name="g", bufs=2))
        vin_p = ctx.enter_context(tc.tile_pool(name="vin", bufs=3))
        vt_p = ctx.enter_context(tc.tile_pool(name="vt", bufs=3))
        vout_p = ctx.enter_context(tc.tile_pool(name="vout", bufs=3))
        ps_tp = ctx.enter_context(tc.tile_pool(name="ps_tp", bufs=2, space="PSUM"))
        ps_mm = ctx.enter_context(tc.tile_pool(name="ps_mm", bufs=2, space="PSUM"))
        ps_at = ctx.enter_context(tc.tile_pool(name="ps_at", bufs=2, space="PSUM"))

        # ---- constants & weights ----
        ident = pp.tile([P, P], F32)
        make_identity(nc, ident)
        ones1 = pp.tile([1, P], F32)
        nc.gpsimd.memset(ones1, 1.0)

        wofft_sb = pp.tile([P, 2, C], F32)
        wattnt_sb = pp.tile([P, 2, 128], F32)
        wvalt_sb = pp.tile([P, 2, C], F32)
        woutt_sb = pp.tile([P, 2, C], F32)
        for k in range(2):
            nc.sync.dma_start(wofft_sb[:, k], wofft[k * P:(k + 1) * P, :])
            nc.sync.dma_start(wattnt_sb[:, k], wattnt[k * P:(k + 1) * P, :])
            nc.sync.dma_start(wvalt_sb[:, k], wvalt[k * P:(k + 1) * P, :])
            nc.sync.dma_start(woutt_sb[:, k], woutt[k * P:(k + 1) * P, :])
        boff_sb = pp.tile([1, C], F32)
        battn_sb = pp.tile([1, 128], F32)
        bval_sb = pp.tile([1, C], F32)
        bout_sb = pp.tile([1, C], F32)
        nc.sync.dma_start(boff_sb[:], boff[:])
        nc.sync.dma_start(battn_sb[:], battn[:])
        nc.sync.dma_start(bval_sb[:], bval[:])
        nc.sync.dma_start(bout_sb[:], bout[:])

        # ---- load q tiles + reference points ----
        qsb = pp.tile([P, NQT, C], F32)
        for qt in range(NQT):
            nc.sync.dma_start(qsb[:, qt], q[qt * P:(qt + 1) * P, :])
        ref_sb = wk.tile([P, NQT, NL, 2], F32, tag="refs")
        nc.sync.dma_start(
            ref_sb.rearrange("p qt l x -> p qt (l x)"),
            ref.rearrange("(qt p) c -> p qt c", p=P)
        )

        # ---- qT via PE transpose, then off/attn projections ----
        qT = pp.tile([P, NQT, C], F32)
        off_sb = wk.tile([P, NQT, C], F32, tag="s8a")
        ssum = wk.tile([P, NQT, NH], F32, tag="ssum")
        rinv = wk.tile([P, NQT, NH], F32, tag="rinv")
        attn_sb = wk.tile([P, NQT, NH, 16], F32, tag="attns")
        for qt in range(NQT):
            pst = ps_tp.tile([P, C], F32, tag="tp")
            for k in range(2):
                nc.tensor.transpose(pst[:, k * P:(k + 1) * P],
                                    qsb[:, qt, k * P:(k + 1) * P], ident[:])
            nc.scalar.copy(qT[:, qt], pst[:])

            psm = ps_mm.tile([P, C], F32, tag="mm")
            nc.tensor.matmul(psm[:], qT[:, qt, 0:P], wofft_sb[:, 0],
                             start=True, stop=False)
            nc.tensor.matmul(psm[:], qT[:, qt, P:C], wofft_sb[:, 1],
                             start=False, stop=False)
            nc.tensor.matmul(psm[:], ones1[:], boff_sb[:],
                             start=False, stop=True)
            nc.scalar.copy(off_sb[:, qt], psm[:])

            psa = ps_at.tile([P, 128], F32, tag="at")
            nc.tensor.matmul(psa[:], qT[:, qt, 0:P], wattnt_sb[:, 0],
                             start=True, stop=False)
            nc.tensor.matmul(psa[:], qT[:, qt, P:C], wattnt_sb[:, 1],
                             start=False, stop=False)
            nc.tensor.matmul(psa[:], ones1[:], battn_sb[:],
                             start=False, stop=True)
            # softmax over the 16 (l,pt) slots per head (no max-sub: |logit|<~3)
            nc.scalar.activation(
                attn_sb[:, qt].rearrange("p h l -> p (h l)"), psa[:], AF.Exp)
            nc.vector.tensor_reduce(
                ssum[:, qt], attn_sb[:, qt], axis=AX.X, op=OP.add)
            nc.vector.reciprocal(rinv[:, qt], ssum[:, qt])
            nc.vector.tensor_tensor(
                attn_sb[:, qt], attn_sb[:, qt],
                _bc(rinv[:, qt], [16]),
                OP.mult)

        # ---- sampling coordinates x,y  (x = ref_x*W + off_x - 0.5) ----
        X = wk.tile([P, 1024], F32, tag="X")     # becomes tx in place
        Y = wk.tile([P, 1024], F32, tag="Y")     # becomes ty in place
        X0 = wk.tile([P, 1024], F32, tag="X0")
        Y0 = wk.tile([P, 1024], F32, tag="Y0")
        U = wk.tile([P, 1024], F32, tag="U")
        SC = wk.tile([P, 1024], F32, tag="SC")
        XI = wk.tile([P, 1024], I32, tag="XI")
        refw = wk.tile([P, NQT, 2], F32, tag="refw")

        def v4(t):  # [128,1024] -> [128, qt, h, l, pt]
            return t.rearrange("p (qt h l pt) -> p qt h l pt",
                               qt=NQT, h=NH, l=NL, pt=NPT)

        off_v = off_sb.rearrange("p qt (h l pt xy) -> p qt h l pt xy",
                                 h=NH, l=NL, pt=NPT, xy=2)
        for l, (Hl, Wl) in enumerate(SPATIAL):
            nc.vector.tensor_scalar(refw[:, :, 0], ref_sb[:, :, l, 0],
                                    float(Wl), -0.5, op0=OP.mult, op1=OP.add)
            nc.vector.tensor_scalar(refw[:, :, 1], ref_sb[:, :, l, 1],
                                    float(Hl), -0.5, op0=OP.mult, op1=OP.add)
            nc.vector.tensor_tensor(v4(X)[:, :, :, l, :],
                                    off_v[:, :, :, l, :, 0],
                                    _bc(refw[:, :, 0], [NH, NPT]), OP.add)
            nc.vector.tensor_tensor(v4(Y)[:, :, :, l, :],
                                    off_v[:, :, :, l, :, 1],
                                    _bc(refw[:, :, 1], [NH, NPT]), OP.add)

        # ---- floor -> X0/Y0 (exact for |x| < 2^23), frac -> X/Y in place ----
        for (A, A0) in ((X, X0), (Y, Y0)):
            nc.vector.tensor_copy(XI[:], A[:])          # f32 -> i32 (round)
            nc.vector.tensor_copy(A0[:], XI[:])         # i32 -> f32
            nc.vector.tensor_tensor(U[:], A0[:], A[:], OP.is_gt)
            nc.vector.tensor_tensor(A0[:], A0[:], U[:], OP.subtract)
            nc.vector.tensor_tensor(A[:], A[:], A0[:], OP.subtract)  # frac

        # ---- x-side: xs = clip(x0,0,W-2), pair weights wp0/wp1 ----
        XS = wk.tile([P, 1024], F32, tag="XI")  # reuse i32 floor scratch slot
        MA = wk.tile([P, 1024], F32, tag="MA")
        MB = wk.tile([P, 1024], F32, tag="MB")
        MC = wk.tile([P, 1024], F32, tag="MC")
        WPX = wk.tile([P, NQT, NH, 16, 2], F32, tag="WPX")
        for l, (Hl, Wl) in enumerate(SPATIAL):
            x0l = v4(X0)[:, :, :, l, :]
            nc.vector.tensor_scalar(v4(XS)[:, :, :, l, :], x0l,
                                    float(Wl - 2), 0.0, op0=OP.min, op1=OP.max)
            nc.vector.tensor_scalar(v4(SC)[:, :, :, l, :], x0l,
                                    0.0, None, op0=OP.is_ge)
            nc.vector.tensor_scalar(v4(MA)[:, :, :, l, :], x0l,
                                    float(Wl - 2), None, op0=OP.is_le)
            nc.vector.tensor_tensor(v4(MA)[:, :, :, l, :],
                                    v4(MA)[:, :, :, l, :],
                                    v4(SC)[:, :, :, l, :], OP.mult)
            nc.vector.tensor_scalar(v4(MB)[:, :, :, l, :], x0l,
                                    -1.0, None, op0=OP.is_equal)
            nc.vector.tensor_scalar(v4(MC)[:, :, :, l, :], x0l,
                                    float(Wl - 1), None, op0=OP.is_equal)
        wpx_v = WPX.rearrange("p qt h l x -> p (qt h l) x")
        nc.vector.tensor_scalar(U[:], X[:], -1.0, 1.0, op0=OP.mult, op1=OP.add)
        nc.vector.tensor_tensor(SC[:], X[:], MB[:], OP.mult)
        nc.vector.tensor_tensor(wpx_v[:, :, 0], U[:], MA[:], OP.mult)
        nc.vector.tensor_tensor(wpx_v[:, :, 0], wpx_v[:, :, 0], SC[:], OP.add)
        nc.vector.tensor_tensor(SC[:], U[:], MC[:], OP.mult)
        nc.vector.tensor_tensor(wpx_v[:, :, 1], X[:], MA[:], OP.mult)
        nc.vector.tensor_tensor(wpx_v[:, :, 1], wpx_v[:, :, 1], SC[:], OP.add)

        # ---- y-side: wy(dy)*my(dy)*attn ----
        WY = wk.tile([P, NQT, NH, 16, 2], F32, tag="WY")
        attn_f = attn_sb.rearrange("p qt h l -> p (qt h l)")
        wy_v = WY.rearrange("p qt h l y -> p (qt h l) y")
        nc.vector.tensor_scalar(U[:], Y[:], -1.0, 1.0, op0=OP.mult, op1=OP.add)
        for dy in range(2):
            for l, (Hl, Wl) in enumerate(SPATIAL):
                y0l = v4(Y0)[:, :, :, l, :]
                nc.vector.tensor_scalar(v4(SC)[:, :, :, l, :], y0l,
                                        float(-dy), None, op0=OP.is_ge)
                nc.vector.tensor_scalar(v4(MA)[:, :, :, l, :], y0l,
                                        float(Hl - 1 - dy), None, op0=OP.is_le)
                nc.vector.tensor_tensor(v4(MA)[:, :, :, l, :],
                                        v4(MA)[:, :, :, l, :],
                                        v4(SC)[:, :, :, l, :], OP.mult)
            nc.vector.tensor_tensor(wy_v[:, :, dy], MA[:],
                                    Y[:] if dy else U[:], OP.mult)
            nc.vector.tensor_tensor(wy_v[:, :, dy], wy_v[:, :, dy],
                                    attn_f, OP.mult)

        # ---- combined weights WF[qt, h, y, lp, xj] = WY * WPX ----
        WF = pp.tile([P, NQT, NH, 2, 16, 2], F32)
        for dy in range(2):
            for xj in range(2):
                nc.vector.tensor_tensor(
                    WF[:, :, :, dy, :, xj],
                    WY[:, :, :, :, dy], WPX[:, :, :, :, xj], OP.mult)

        # ---- gather row indices IDX2[qt, h, lp, y] = loff + yc*W + xs ----
        IDX2 = pp.tile([P, NQT, NH, 16, 2], I32)
        IDXF = wk.tile([P, 2048], F32, tag="s8a")  # reuse off_sb slot
        idxf_v = IDXF.rearrange("p (qt h l pt y) -> p qt h l pt y",
                                qt=NQT, h=NH, l=NL, pt=NPT, y=2)
        for l, (Hl, Wl) in enumerate(SPATIAL):
            for dy in range(2):
                sl = v4(SC)[:, :, :, l, :]
                nc.vector.tensor_scalar(sl, v4(Y0)[:, :, :, l, :],
                                        float(dy), 0.0, op0=OP.add, op1=OP.max)
                nc.vector.tensor_scalar(sl, sl, float(Hl - 1), None, op0=OP.min)
                nc.vector.tensor_scalar(sl, sl, float(Wl), float(LOFF[l]),
                                        op0=OP.mult, op1=OP.add)
                nc.vector.tensor_tensor(idxf_v[:, :, :, l, :, dy], sl,
                                        v4(XS)[:, :, :, l, :], OP.add)
        nc.vector.tensor_copy(IDX2.rearrange("p qt h l y -> p (qt h l y)"),
                              IDXF[:])

        # ---- value projection: val = v @ W_val.T + b_val ----
        for ch in range(NCH):
            vin = vin_p.tile([P, C], F32, tag="vin")
            nc.sync.dma_start(vin[:], v[ch * P:(ch + 1) * P, :])
            pst = ps_tp.tile([P, C], F32, tag="tp")
            for k in range(2):
                nc.tensor.transpose(pst[:, k * P:(k + 1) * P],
                                    vin[:, k * P:(k + 1) * P], ident[:])
            vt = vt_p.tile([P, C], F32, tag="vt")
            nc.scalar.copy(vt[:], pst[:])
            psv = ps_mm.tile([P, C], F32, tag="mm")
            nc.tensor.matmul(psv[:], vt[:, 0:P], wvalt_sb[:, 0],
                             start=True, stop=False)
            nc.tensor.matmul(psv[:], vt[:, P:C], wvalt_sb[:, 1],
                             start=False, stop=False)
            nc.tensor.matmul(psv[:], ones1[:], bval_sb[:],
                             start=False, stop=True)
            vout = vout_p.tile([P, C], F32, tag="vout")
            nc.scalar.copy(vout[:], psv[:])
            nc.sync.dma_start(val[ch * P:(ch + 1) * P, :], vout[:])

        # ---- gathers + weighted sum ----
        pre = pp.tile([P, NQT, NH, HD], F32)
        TMP = wk.tile([P, 16, 2, HD], F32, tag="X")    # reuse tx slot
        TMP2 = wk.tile([P, HD], F32, tag="Y")          # reuse ty slot
        for h in range(NH):
            for qt in range(NQT):
                for dy in range(2):
                    G = gpool.tile([P, 16, SPAN], F32, tag="G")
                    for lp in range(16):
                        nc.gpsimd.indirect_dma_start(
                            out=G[:, lp],
                            out_offset=None,
                            in_=val,
                            in_offset=IndirectOffsetOnAxis(
                                ap=IDX2[:, qt, h, lp, dy:dy + 1], axis=0),
                            element_offset=h * HD,
                        )
                    # tmp[lp, xj, c] = G[lp, xj*256 + c] * WF[qt,h,dy,lp,xj]
                    for xj in range(2):
                        gsl = bass.AP(G.tensor, G.offset + xj * C,
                                      [list(G.ap[0]), [SPAN, 16], [1, HD]])
                        nc.vector.scalar_tensor_tensor(
                            TMP[:, :, xj, :], gsl, 1.0,
                            _bc(WF[:, qt, h, dy, :, xj], [HD]),
                            op0=OP.mult, op1=OP.mult)
                    red_out = pre[:, qt, h] if dy == 0 else TMP2[:]
                    nc.vector.tensor_reduce(
                        red_out,
                        TMP.rearrange("p l x d -> p d l x"),
                        axis=AX.XY, op=OP.add)
                    if dy == 1:
                        nc.vector.tensor_tensor(pre[:, qt, h], pre[:, qt, h],
                                                TMP2[:], OP.add)

        # ---- output projection + bias + residual ----
        for qt in range(NQT):
            pst = ps_tp.tile([P, C], F32, tag="tp")
            pre_f = pre[:, qt].rearrange("p h d -> p (h d)")
            for k in range(2):
                nc.tensor.transpose(pst[:, k * P:(k + 1) * P],
                                    pre_f[:, k * P:(k + 1) * P], ident[:])
            pT = vt_p.tile([P, C], F32, tag="vt")
            nc.scalar.copy(pT[:], pst[:])
            pso = ps_mm.tile([P, C], F32, tag="mm")
            nc.tensor.matmul(pso[:], pT[:, 0:P], woutt_sb[:, 0],
                             start=True, stop=False)
            nc.tensor.matmul(pso[:], pT[:, P:C], woutt_sb[:, 1],
                             start=False, stop=False)
            nc.tensor.matmul(pso[:], ones1[:], bout_sb[:],
                             start=False, stop=True)
            osb = vout_p.tile([P, C], F32, tag="vout")
            nc.vector.tensor_tensor(osb[:], pso[:], qsb[:, qt], OP.add)
            nc.sync.dma_start(out[qt * P:(qt + 1) * P, :], osb[:])

    nc.compile()
    return nc


def _make_in_maps(query, value, reference_points, W_off, b_off, W_attn,
                  b_attn, W_val, b_val, W_out, b_out):
    wofft = np.ascontiguousarray(W_off.T)
    wattnt = np.ascontiguousarray(W_attn.T)
    wvalt = np.ascontiguousarray(W_val.T)
    woutt = np.ascontiguousarray(W_out.T)
    shared = {
        "wofft": wofft, "boff": b_off.reshape(1, C),
        "wattnt": wattnt, "battn": b_attn.reshape(1, 128),
        "wvalt": wvalt, "bval": b_val.reshape(1, C),
        "woutt": woutt, "bout": b_out.reshape(1, C),
    }
    in_maps = []
    for b in range(N_CORES):
        qp = np.zeros((QP, C), np.float32)
        qp[:NQ] = query[:, b, :]
        refp = np.full((QP, NL * 2), 0.5, np.float32)
        refp[:NQ] = reference_points[b].reshape(NQ, NL * 2)
        in_maps.append({
            "q": qp,
            "v": np.ascontiguousarray(value[:, b, :]),
            "ref": refp,
            **shared,
        })
    return in_maps


def _build_exec(nc):
    """Memoized jitted SPMD callable mirroring bass2jax.run_bass_via_pjrt's
    multi-core path, so repeat calls skip retracing and inputs can be staged
    on device for timing."""
    import jax
    from jax.experimental.shard_map import shard_map
    from jax.sharding import Mesh, PartitionSpec
    from concourse import bass2jax, mybir as mb

    bass2jax.install_neuronx_cc_hook()
    in_names, out_names, out_avals, zero_outs = [], [], [], []
    partition_name = (nc.partition_id_tensor.name
                      if nc.partition_id_tensor else None)
    for alloc in nc.m.functions[0].allocations:
        if not isinstance(alloc, mb.MemoryLocationSet):
            continue
        name = alloc.memorylocations[0].name
        if alloc.kind == "ExternalInput":
            if name != partition_name:
                in_names.append(name)
        elif alloc.kind == "ExternalOutput":
            shape = tuple(alloc.tensor_shape)
            dtype = mb.dt.np(alloc.dtype)
            out_names.append(name)
            out_avals.append(jax.core.ShapedArray(shape, dtype))
            zero_outs.append(np.zeros(shape, dtype))
    n_params = len(in_names)
    all_names = list(in_names) + list(out_names)
    if partition_name is not None:
        all_names.append(partition_name)
    donate = tuple(range(n_params, n_params + len(out_names)))

    def _body(*args):
        operands = list(args)
        if partition_name is not None:
            operands.append(bass2jax.partition_id_tensor())
        return tuple(bass2jax._bass_exec_p.bind(
            *operands,
            out_avals=tuple(out_avals),
            in_names=tuple(all_names),
            out_names=tuple(out_names),
            lowering_input_output_aliases=(),
            sim_require_finite=True,
            sim_require_nnan=True,
            nc=nc,
        ))

    devices = jax.devices()[:N_CORES]
    mesh = Mesh(np.asarray(devices), ("core",))
    in_specs = (PartitionSpec("core"),) * (n_params + len(out_names))
    out_specs = (PartitionSpec("core"),) * len(out_names)
    fn = jax.jit(
        shard_map(_body, mesh=mesh, in_specs=in_specs, out_specs=out_specs,
                  check_rep=False),
        donate_argnums=donate, keep_unused=True)

    return {
        "fn": fn, "mesh": mesh, "in_names": in_names,
        "out_names": out_names, "zero_outs": zero_outs,
        "out_avals": out_avals,
    }


def _exec(in_maps, timeit=0):
    """Run the compiled kernel on 8 cores. Returns (results, best_ns).
    timeit>0: additionally re-run the executable with device-staged inputs
    `timeit` times and report the best wall-clock ns."""
    import jax, time
    from jax.sharding import NamedSharding, PartitionSpec

    if "exec" not in _COMPILED:
        _COMPILED["exec"] = _build_exec(_COMPILED["nc"])
    ex = _COMPILED["exec"]
    fn, mesh = ex["fn"], ex["mesh"]
    concat_in = [
        np.concatenate([np.asarray(m[name]) for m in in_maps], axis=0)
        for name in ex["in_names"]
    ]
    concat_zeros = [
        np.zeros((N_CORES * z.shape[0], *z.shape[1:]), z.dtype)
        for z in ex["zero_outs"]
    ]
    out_arrs = fn(*concat_in, *concat_zeros)
    jax.block_until_ready(out_arrs)
    results = [
        {name: np.asarray(out_arrs[i]).reshape(
            N_CORES, *ex["out_avals"][i].shape)[c]
         for i, name in enumerate(ex["out_names"])}
        for c in range(N_CORES)
    ]
    best_ns = None
    if timeit:
        shard = NamedSharding(mesh, PartitionSpec("core"))
        staged = [jax.device_put(x, shard) for x in concat_in]
        jax.block_until_ready(staged)
        for _ in range(timeit):
            zo = [jax.device_put(z, shard) for z in concat_zeros]
            jax.block_until_ready(zo)
            t0 = time.perf_counter()
            o = fn(*staged, *zo)
            jax.block_until_ready(o)
            dt = time.perf_counter() - t0
            best_ns = dt * 1e9 if best_ns is None else min(best_ns, dt * 1e9)
    return results, best_ns


def hw_exec_ns(in_maps, n_long=16, repeats=3):
    """Per-NEFF silicon execution time, measured as the marginal wall-clock
    per execution when n_long executions are dispatched asynchronously
    (PJRT serializes them on the device stream):
        t_neff = (wall(n_long) - wall(1)) / (n_long - 1)
    This removes the constant axon/PJRT dispatch overhead (~68 ms) that a
    single blocking call would include. Returns (t_neff_ns, wall1, wallN)."""
    import jax, time
    from jax.sharding import NamedSharding, PartitionSpec

    if "exec" not in _COMPILED:
        _COMPILED["exec"] = _build_exec(_COMPILED["nc"])
    ex = _COMPILED["exec"]
    fn = ex["fn"]
    shard = NamedSharding(ex["mesh"], PartitionSpec("core"))
    staged = [
        jax.device_put(
            np.concatenate([np.asarray(m[name]) for m in in_maps], axis=0),
            shard)
        for name in ex["in_names"]
    ]
    jax.block_until_ready(staged)
    cz = [np.zeros((N_CORES * z.shape[0], *z.shape[1:]), z.dtype)
          for z in ex["zero_outs"]]

    def run_n(n):
        zos = [[jax.device_put(z, shard) for z in cz] for _ in range(n)]
        for zo in zos:
            jax.block_until_ready(zo)
        t0 = time.perf_counter()
        outs = [fn(*staged, *zo) for zo in zos]
        jax.block_until_ready(outs)
        return time.perf_counter() - t0

    run_n(1)
    run_n(2)  # warm-up
    t1 = min(run_n(1) for _ in range(repeats))
    tn = min(run_n(n_long) for _ in range(repeats))
    return (tn - t1) / (n_long - 1) * 1e9, t1 * 1e9, tn * 1e9


def kernel(**inputs):
    query = np.asarray(inputs["query"], np.float32)
    value = np.asarray(inputs["value"], np.float32)
    reference_points = np.asarray(inputs["reference_points"], np.float32)
    W_off = np.asarray(inputs["W_off"], np.float32)
    b_off = np.asarray(inputs["b_off"], np.float32)
    W_attn = np.asarray(inputs["W_attn"], np.float32)
    b_attn = np.asarray(inputs["b_attn"], np.float32)
    W_val = np.asarray(inputs["W_val"], np.float32)
    b_val = np.asarray(inputs["b_val"], np.float32)
    W_out = np.asarray(inputs["W_out"], np.float32)
    b_out = np.asarray(inputs["b_out"], np.float32)

    if "nc" not in _COMPILED:
        _COMPILED["nc"] = _build_nc()
    nc = _COMPILED["nc"]

    in_maps = _make_in_maps(query, value, reference_points, W_off, b_off,
                            W_attn, b_attn, W_val, b_val, W_out, b_out)
    results, _ = _exec(in_maps)
    outs = [results[b]["out"][:NQ] for b in range(N_CORES)]
    return np.stack(outs, axis=1).astype(np.float32)


# revision 29
# speedup vs baseline: 4530.6878x; 1.1187x over previous
"""Multi-scale deformable attention — TRN2 Bass kernel, fully on-device.

Sharding: data-parallel over batch (bs=8 -> one batch element per NeuronCore).
Each core runs the ENTIRE computation for its batch element in one NEFF:
  1. value projection (v @ W_val.T + b_val) -> val scratch in HBM
     (v transposed on-device via PE identity transposes)
  2. query-side: off/attn projections (PE), softmax (ACT+DVE), sampling
     locations, bilinear weights and gather row indices (DVE)
  3. 2048 per-partition indirect-DMA gathers (gpsimd SWDGE): each pulls a
     288-element contiguous span val[row, h*32 : ...] covering both
     x-corners (row, row+1) of one (head, level-point, y-corner, q-tile)
  4. weighted sum over (level-point, y-corner, x-corner) on DVE -> pre
  5. output projection + bias + residual (PE) -> out
Host only pads/slices per-core arrays and re-assembles the full output.

The x-corner pair is folded into one gather: row = loff + yc*W + xs with
xs = clip(x0, 0, W-2); pair weights
  wp0 = (1-tx)*[0<=x0<=W-2] + tx*[x0==-1]
  wp1 = tx*[0<=x0<=W-2] + (1-tx)*[x0==W-1]
reproduce the reference's per-corner valid masks exactly.
"""
import sys

for _p in ("/opt/trn_rl_repo", "/opt/trn_rl_repo/concourse"):
    if _p not in sys.path:
        sys.path.insert(0, _p)

import numpy as np
from contextlib import ExitStack

import concourse.bass as bass
import concourse.tile as tile
from concourse import bacc, mybir
from concourse.bass import IndirectOffsetOnAxis
from concourse.bass_utils import run_bass_kernel_spmd
from concourse.masks import make_identity

F32 = mybir.dt.float32
I32 = mybir.dt.int32
AF = mybir.ActivationFunctionType
OP = mybir.AluOpType
AX = mybir.AxisListType

# Static problem config (matches reference.py / spec.json)
SPATIAL = [(128, 128), (64, 64), (32, 32), (16, 16)]
LOFF = [0, 16384, 20480, 21504]
NH, NL, NPT, C, HD = 8, 4, 4, 256, 32
NQ, QP, BS, NV = 900, 1024, 8, 21760
P = 128
NQT = QP // P          # 8 query tiles
NCH = NV // P          # 170 value chunks
SPAN = C + HD          # 288: gathered span covers (row, h*32) .. (row+1, h*32+32)
N_CORES = 8

_COMPILED = {}


def _bc(ap, sizes):
    """Append stride-0 broadcast dims to an AP."""
    return ap.to_broadcast(list(ap.shape) + [int(s) for s in sizes])


def _build_nc():
    nc = bacc.Bacc("TRN2", target_bir_lowering=False, debug=False)
    q = nc.dram_tensor("q", [QP, C], F32, kind="ExternalInput").ap()
    v = nc.dram_tensor("v", [NV, C], F32, kind="ExternalInput").ap()
    ref = nc.dram_tensor("ref", [QP, NL * 2], F32, kind="ExternalInput").ap()
    wofft = nc.dram_tensor("wofft", [C, C], F32, kind="ExternalInput").ap()
    boff = nc.dram_tensor("boff", [1, C], F32, kind="ExternalInput").ap()
    wattnt = nc.dram_tensor("wattnt", [C, 128], F32, kind="ExternalInput").ap()
    battn = nc.dram_tensor("battn", [1, 128], F32, kind="ExternalInput").ap()
    wvalt = nc.dram_tensor("wvalt", [C, C], F32, kind="ExternalInput").ap()
    bval = nc.dram_tensor("bval", [1, C], F32, kind="ExternalInput").ap()
    woutt = nc.dram_tensor("woutt", [C, C], F32, kind="ExternalInput").ap()
    bout = nc.dram_tensor("bout", [1, C], F32, kind="ExternalInput").ap()
    out = nc.dram_tensor("out", [QP, C], F32, kind="ExternalOutput").ap()
    # per-level value scratch: gathers for level l only depend on val_l, so
    # level-3/2/1 gathers start while level-0 projection is still running
    # (span reads end exactly at the level's last row: xs <= W-2)
    val_l = [nc.dram_tensor(f"valbuf{l}", [Hl * Wl, C], F32, kind="Internal").ap()
             for l, (Hl, Wl) in enumerate(SPATIAL)]

    with tile.TileContext(nc) as tc, ExitStack() as ctx:
        pp = ctx.enter_context(tc.tile_pool(name="pers", bufs=1))
        wk = ctx.enter_context(tc.tile_pool(name="wk", bufs=1))
        gpool = ctx.enter_context(tc.tile_pool(name="g", bufs=4))
        vin_p = ctx.enter_context(tc.tile_pool(name="vin", bufs=3))
        vt_p = ctx.enter_context(tc.tile_pool(name="vt", bufs=3))
        vout_p = ctx.enter_context(tc.tile_pool(name="vout", bufs=3))
        ps_tp = ctx.enter_context(tc.tile_pool(name="ps_tp", bufs=2, space="PSUM"))
        ps_mm = ctx.enter_context(tc.tile_pool(name="ps_mm", bufs=2, space="PSUM"))
        ps_at = ctx.enter_context(tc.tile_pool(name="ps_at", bufs=2, space="PSUM"))

        # ---- constants & weights ----
        ident = pp.tile([P, P], F32)
        make_identity(nc, ident)
        ones1 = pp.tile([1, P], F32)
        nc.gpsimd.memset(ones1, 1.0)

        wofft_sb = pp.tile([P, 2, C], F32)
        wattnt_sb = pp.tile([P, 2, 128], F32)
        wvalt_sb = pp.tile([P, 2, C], F32)
        woutt_sb = pp.tile([P, 2, C], F32)
        for k in range(2):
            nc.sync.dma_start(wofft_sb[:, k], wofft[k * P:(k + 1) * P, :])
            nc.sync.dma_start(wattnt_sb[:, k], wattnt[k * P:(k + 1) * P, :])
            nc.sync.dma_start(wvalt_sb[:, k], wvalt[k * P:(k + 1) * P, :])
            nc.sync.dma_start(woutt_sb[:, k], woutt[k * P:(k + 1) * P, :])
        boff_sb = pp.tile([1, C], F32)
        battn_sb = pp.tile([1, 128], F32)
        bval_sb = pp.tile([1, C], F32)
        bout_sb = pp.tile([1, C], F32)
        nc.sync.dma_start(boff_sb[:], boff[:])
        nc.sync.dma_start(battn_sb[:], battn[:])
        nc.sync.dma_start(bval_sb[:], bval[:])
        nc.sync.dma_start(bout_sb[:], bout[:])

        # ---- load q tiles + reference points ----
        qsb = pp.tile([P, NQT, C], F32)
        for qt in range(NQT):
            nc.sync.dma_start(qsb[:, qt], q[qt * P:(qt + 1) * P, :])
        ref_sb = wk.tile([P, NQT, NL, 2], F32, tag="refs")
        nc.sync.dma_start(
            ref_sb.rearrange("p qt l x -> p qt (l x)"),
            ref.rearrange("(qt p) c -> p qt c", p=P)
        )

        # ---- qT via PE transpose, then off/attn projections ----
        qT = pp.tile([P, NQT, C], F32)
        off_sb = wk.tile([P, NQT, C], F32, tag="s8a")
        ssum = wk.tile([P, NQT, NH], F32, tag="ssum")
        rinv = wk.tile([P, NQT, NH], F32, tag="rinv")
        attn_sb = wk.tile([P, NQT, NH, 16], F32, tag="attns")
        for qt in range(NQT):
            pst = ps_tp.tile([P, C], F32, tag="tp")
            for k in range(2):
                nc.tensor.transpose(pst[:, k * P:(k + 1) * P],
                                    qsb[:, qt, k * P:(k + 1) * P], ident[:])
            nc.scalar.copy(qT[:, qt], pst[:])

            psm = ps_mm.tile([P, C], F32, tag="mm")
            nc.tensor.matmul(psm[:], qT[:, qt, 0:P], wofft_sb[:, 0],
                             start=True, stop=False)
            nc.tensor.matmul(psm[:], qT[:, qt, P:C], wofft_sb[:, 1],
                             start=False, stop=False)
            nc.tensor.matmul(psm[:], ones1[:], boff_sb[:],
                             start=False, stop=True)
            nc.scalar.copy(off_sb[:, qt], psm[:])

            psa = ps_at.tile([P, 128], F32, tag="at")
            nc.tensor.matmul(psa[:], qT[:, qt, 0:P], wattnt_sb[:, 0],
                             start=True, stop=False)
            nc.tensor.matmul(psa[:], qT[:, qt, P:C], wattnt_sb[:, 1],
                             start=False, stop=False)
            nc.tensor.matmul(psa[:], ones1[:], battn_sb[:],
                             start=False, stop=True)
            # softmax over the 16 (l,pt) slots per head (no max-sub: |logit|<~3)
            nc.scalar.activation(
                attn_sb[:, qt].rearrange("p h l -> p (h l)"), psa[:], AF.Exp)
            nc.vector.tensor_reduce(
                ssum[:, qt], attn_sb[:, qt], axis=AX.X, op=OP.add)
            nc.vector.reciprocal(rinv[:, qt], ssum[:, qt])
            nc.vector.tensor_tensor(
                attn_sb[:, qt], attn_sb[:, qt],
                _bc(rinv[:, qt], [16]),
                OP.mult)

        # ---- sampling coordinates x,y  (x = ref_x*W + off_x - 0.5) ----
        X = wk.tile([P, 1024], F32, tag="X")     # becomes tx in place
        Y = wk.tile([P, 1024], F32, tag="Y")     # becomes ty in place
        X0 = wk.tile([P, 1024], F32, tag="X0")
        Y0 = wk.tile([P, 1024], F32, tag="Y0")
        U = wk.tile([P, 1024], F32, tag="U")
        SC = wk.tile([P, 1024], F32, tag="SC")
        XI = wk.tile([P, 1024], I32, tag="XI")
        refw = wk.tile([P, NQT, 2], F32, tag="refw")

        def v4(t):  # [128,1024] -> [128, qt, h, l, pt]
            return t.rearrange("p (qt h l pt) -> p qt h l pt",
                               qt=NQT, h=NH, l=NL, pt=NPT)

        off_v = off_sb.rearrange("p qt (h l pt xy) -> p qt h l pt xy",
                                 h=NH, l=NL, pt=NPT, xy=2)
        for l, (Hl, Wl) in enumerate(SPATIAL):
            nc.vector.tensor_scalar(refw[:, :, 0], ref_sb[:, :, l, 0],
                                    float(Wl), -0.5, op0=OP.mult, op1=OP.add)
            nc.vector.tensor_scalar(refw[:, :, 1], ref_sb[:, :, l, 1],
                                    float(Hl), -0.5, op0=OP.mult, op1=OP.add)
            nc.vector.tensor_tensor(v4(X)[:, :, :, l, :],
                                    off_v[:, :, :, l, :, 0],
                                    _bc(refw[:, :, 0], [NH, NPT]), OP.add)
            nc.vector.tensor_tensor(v4(Y)[:, :, :, l, :],
                                    off_v[:, :, :, l, :, 1],
                                    _bc(refw[:, :, 1], [NH, NPT]), OP.add)

        # ---- floor -> X0/Y0 (exact for |x| < 2^23), frac -> X/Y in place ----
        for (A, A0) in ((X, X0), (Y, Y0)):
            nc.vector.tensor_copy(XI[:], A[:])          # f32 -> i32 (round)
            nc.vector.tensor_copy(A0[:], XI[:])         # i32 -> f32
            nc.vector.tensor_tensor(U[:], A0[:], A[:], OP.is_gt)
            nc.vector.tensor_tensor(A0[:], A0[:], U[:], OP.subtract)
            nc.vector.tensor_tensor(A[:], A[:], A0[:], OP.subtract)  # frac

        # ---- x-side: xs = clip(x0,0,W-2), pair weights wp0/wp1 ----
        XS = wk.tile([P, 1024], F32, tag="XI")  # reuse i32 floor scratch slot
        MA = wk.tile([P, 1024], F32, tag="MA")
        MB = wk.tile([P, 1024], F32, tag="MB")
        MC = wk.tile([P, 1024], F32, tag="MC")
        WPX = wk.tile([P, NQT, NH, 16, 2], F32, tag="WPX")
        for l, (Hl, Wl) in enumerate(SPATIAL):
            x0l = v4(X0)[:, :, :, l, :]
            nc.vector.tensor_scalar(v4(XS)[:, :, :, l, :], x0l,
                                    float(Wl - 2), 0.0, op0=OP.min, op1=OP.max)
            nc.vector.tensor_scalar(v4(SC)[:, :, :, l, :], x0l,
                                    0.0, None, op0=OP.is_ge)
            nc.vector.tensor_scalar(v4(MA)[:, :, :, l, :], x0l,
                                    float(Wl - 2), None, op0=OP.is_le)
            nc.vector.tensor_tensor(v4(MA)[:, :, :, l, :],
                                    v4(MA)[:, :, :, l, :],
                                    v4(SC)[:, :, :, l, :], OP.mult)
            nc.vector.tensor_scalar(v4(MB)[:, :, :, l, :], x0l,
                                    -1.0, None, op0=OP.is_equal)
            nc.vector.tensor_scalar(v4(MC)[:, :, :, l, :], x0l,
                                    float(Wl - 1), None, op0=OP.is_equal)
        wpx_v = WPX.rearrange("p qt h l x -> p (qt h l) x")
        nc.vector.tensor_scalar(U[:], X[:], -1.0, 1.0, op0=OP.mult, op1=OP.add)
        nc.vector.tensor_tensor(SC[:], X[:], MB[:], OP.mult)
        nc.vector.tensor_tensor(wpx_v[:, :, 0], U[:], MA[:], OP.mult)
        nc.vector.tensor_tensor(wpx_v[:, :, 0], wpx_v[:, :, 0], SC[:], OP.add)
        nc.vector.tensor_tensor(SC[:], U[:], MC[:], OP.mult)
        nc.vector.tensor_tensor(wpx_v[:, :, 1], X[:], MA[:], OP.mult)
        nc.vector.tensor_tensor(wpx_v[:, :, 1], wpx_v[:, :, 1], SC[:], OP.add)

        # ---- y-side: wy(dy)*my(dy)*attn ----
        WY = wk.tile([P, NQT, NH, 16, 2], F32, tag="WY")
        attn_f = attn_sb.rearrange("p qt h l -> p (qt h l)")
        wy_v = WY.rearrange("p qt h l y -> p (qt h l) y")
        nc.vector.tensor_scalar(U[:], Y[:], -1.0, 1.0, op0=OP.mult, op1=OP.add)
        for dy in range(2):
            for l, (Hl, Wl) in enumerate(SPATIAL):
                y0l = v4(Y0)[:, :, :, l, :]
                nc.vector.tensor_scalar(v4(SC)[:, :, :, l, :], y0l,
                                        float(-dy), None, op0=OP.is_ge)
                nc.vector.tensor_scalar(v4(MA)[:, :, :, l, :], y0l,
                                        float(Hl - 1 - dy), None, op0=OP.is_le)
                nc.vector.tensor_tensor(v4(MA)[:, :, :, l, :],
                                        v4(MA)[:, :, :, l, :],
                                        v4(SC)[:, :, :, l, :], OP.mult)
            nc.vector.tensor_tensor(wy_v[:, :, dy], MA[:],
                                    Y[:] if dy else U[:], OP.mult)
            nc.vector.tensor_tensor(wy_v[:, :, dy], wy_v[:, :, dy],
                                    attn_f, OP.mult)

        # ---- combined weights WF[qt, h, y, lp, xj] = WY * WPX ----
        WF = pp.tile([P, NQT, NH, 2, 16, 2], F32)
        for dy in range(2):
            for xj in range(2):
                nc.vector.tensor_tensor(
                    WF[:, :, :, dy, :, xj],
                    WY[:, :, :, :, dy], WPX[:, :, :, :, xj], OP.mult)

        # ---- gather row indices IDX2[qt, h, lp, y] = loff + yc*W + xs ----
        IDX2 = pp.tile([P, NQT, NH, 16, 2], I32)
        IDXF = wk.tile([P, 2048], F32, tag="s8a")  # reuse off_sb slot
        idxf_v = IDXF.rearrange("p (qt h l pt y) -> p qt h l pt y",
                                qt=NQT, h=NH, l=NL, pt=NPT, y=2)
        for l, (Hl, Wl) in enumerate(SPATIAL):
            for dy in range(2):
                sl = v4(SC)[:, :, :, l, :]
                nc.vector.tensor_scalar(sl, v4(Y0)[:, :, :, l, :],
                                        float(dy), 0.0, op0=OP.add, op1=OP.max)
                nc.vector.tensor_scalar(sl, sl, float(Hl - 1), None, op0=OP.min)
                nc.vector.tensor_scalar(sl, sl, float(Wl), 0.0,
                                        op0=OP.mult, op1=OP.add)
                nc.vector.tensor_tensor(idxf_v[:, :, :, l, :, dy], sl,
                                        v4(XS)[:, :, :, l, :], OP.add)
        nc.vector.tensor_copy(IDX2.rearrange("p qt h l y -> p (qt h l y)"),
                              IDXF[:])

        # ---- value projection: val_l = (v @ W_val.T + b_val)[level slice] ----
        # levels 3,2,1 first (10 chunks) so their gathers can start while
        # level 0's 128 chunks are still being projected
        for l in (3, 2, 1, 0):
            for lc in range(SPATIAL[l][0] * SPATIAL[l][1] // P):
                gr = LOFF[l] + lc * P
                vin = vin_p.tile([P, C], F32, tag="vin")
                nc.sync.dma_start(vin[:], v[gr:gr + P, :])
                pst = ps_tp.tile([P, C], F32, tag="tp")
                for k in range(2):
                    nc.tensor.transpose(pst[:, k * P:(k + 1) * P],
                                        vin[:, k * P:(k + 1) * P], ident[:])
                vt = vt_p.tile([P, C], F32, tag="vt")
                nc.scalar.copy(vt[:], pst[:])
                psv = ps_mm.tile([P, C], F32, tag="mm")
                nc.tensor.matmul(psv[:], vt[:, 0:P], wvalt_sb[:, 0],
                                 start=True, stop=False)
                nc.tensor.matmul(psv[:], vt[:, P:C], wvalt_sb[:, 1],
                                 start=False, stop=False)
                nc.tensor.matmul(psv[:], ones1[:], bval_sb[:],
                                 start=False, stop=True)
                vout = vout_p.tile([P, C], F32, tag="vout")
                nc.scalar.copy(vout[:], psv[:])
                nc.sync.dma_start(val_l[l][lc * P:(lc + 1) * P, :], vout[:])

        # ---- gathers + weighted sum (level-major: l=3 gathers first) ----
        pre = pp.tile([P, NQT, NH, HD], F32)
        TMP = wk.tile([P, 4, 2, HD], F32, tag="X")     # reuse tx slot
        TMP2 = wk.tile([P, HD], F32, tag="Y")          # reuse ty slot
        for l in (3, 2, 1, 0):
            for h in range(NH):
                for qt in range(NQT):
                    for dy in range(2):
                        G = gpool.tile([P, 4, SPAN], F32, tag="G")
                        for pt in range(4):
                            nc.gpsimd.indirect_dma_start(
                                out=G[:, pt],
                                out_offset=None,
                                in_=val_l[l],
                                in_offset=IndirectOffsetOnAxis(
                                    ap=IDX2[:, qt, h, l * 4 + pt, dy:dy + 1],
                                    axis=0),
                                element_offset=h * HD,
                            )
                        # tmp[pt, xj, c] = G[pt, xj*256+c] * WF[qt,h,dy,lp,xj]
                        for xj in range(2):
                            gsl = bass.AP(G.tensor, G.offset + xj * C,
                                          [list(G.ap[0]), [SPAN, 4], [1, HD]])
                            nc.vector.scalar_tensor_tensor(
                                TMP[:, :, xj, :], gsl, 1.0,
                                _bc(WF[:, qt, h, dy, l * 4:(l + 1) * 4, xj],
                                    [HD]),
                                op0=OP.mult, op1=OP.mult)
                        first = (l == 3 and dy == 0)
                        red_out = pre[:, qt, h] if first else TMP2[:]
                        nc.vector.tensor_reduce(
                            red_out,
                            TMP.rearrange("p l x d -> p d l x"),
                            axis=AX.XY, op=OP.add)
                        if not first:
                            nc.vector.tensor_tensor(
                                pre[:, qt, h], pre[:, qt, h], TMP2[:], OP.add)

        # ---- output projection + bias + residual ----
        for qt in range(NQT):
            pst = ps_tp.tile([P, C], F32, tag="tp")
            pre_f = pre[:, qt].rearrange("p h d -> p (h d)")
            for k in range(2):
                nc.tensor.transpose(pst[:, k * P:(k + 1) * P],
                                    pre_f[:, k * P:(k + 1) * P], ident[:])
            pT = vt_p.tile([P, C], F32, tag="vt")
            nc.scalar.copy(pT[:], pst[:])
            pso = ps_mm.tile([P, C], F32, tag="mm")
            nc.tensor.matmul(pso[:], pT[:, 0:P], woutt_sb[:, 0],
                             start=True, stop=False)
            nc.tensor.matmul(pso[:], pT[:, P:C], woutt_sb[:, 1],
                             start=False, stop=False)
            nc.tensor.matmul(pso[:], ones1[:], bout_sb[:],
                             start=False, stop=True)
            osb = vout_p.tile([P, C], F32, tag="vout")
            nc.vector.tensor_tensor(osb[:], pso[:], qsb[:, qt], OP.add)
            nc.sync.dma_start(out[qt * P:(qt + 1) * P, :], osb[:])

    nc.compile()
    return nc


def _make_in_maps(query, value, reference_points, W_off, b_off, W_attn,
                  b_attn, W_val, b_val, W_out, b_out):
    wofft = np.ascontiguousarray(W_off.T)
    wattnt = np.ascontiguousarray(W_attn.T)
    wvalt = np.ascontiguousarray(W_val.T)
    woutt = np.ascontiguousarray(W_out.T)
    shared = {
        "wofft": wofft, "boff": b_off.reshape(1, C),
        "wattnt": wattnt, "battn": b_attn.reshape(1, 128),
        "wvalt": wvalt, "bval": b_val.reshape(1, C),
        "woutt": woutt, "bout": b_out.reshape(1, C),
    }
    in_maps = []
    for b in range(N_CORES):
        qp = np.zeros((QP, C), np.float32)
        qp[:NQ] = query[:, b, :]
        refp = np.full((QP, NL * 2), 0.5, np.float32)
        refp[:NQ] = reference_points[b].reshape(NQ, NL * 2)
        in_maps.append({
            "q": qp,
            "v": np.ascontiguousarray(value[:, b, :]),
            "ref": refp,
            **shared,
        })
    return in_maps


def _build_exec(nc):
    """Memoized jitted SPMD callable mirroring bass2jax.run_bass_via_pjrt's
    multi-core path, so repeat calls skip retracing and inputs can be staged
    on device for timing."""
    import jax
    from jax.experimental.shard_map import shard_map
    from jax.sharding import Mesh, PartitionSpec
    from concourse import bass2jax, mybir as mb

    bass2jax.install_neuronx_cc_hook()
    in_names, out_names, out_avals, zero_outs = [], [], [], []
    partition_name = (nc.partition_id_tensor.name
                      if nc.partition_id_tensor else None)
    for alloc in nc.m.functions[0].allocations:
        if not isinstance(alloc, mb.MemoryLocationSet):
            continue
        name = alloc.memorylocations[0].name
        if alloc.kind == "ExternalInput":
            if name != partition_name:
                in_names.append(name)
        elif alloc.kind == "ExternalOutput":
            shape = tuple(alloc.tensor_shape)
            dtype = mb.dt.np(alloc.dtype)
            out_names.append(name)
            out_avals.append(jax.core.ShapedArray(shape, dtype))
            zero_outs.append(np.zeros(shape, dtype))
    n_params = len(in_names)
    all_names = list(in_names) + list(out_names)
    if partition_name is not None:
        all_names.append(partition_name)
    donate = tuple(range(n_params, n_params + len(out_names)))

    def _body(*args):
        operands = list(args)
        if partition_name is not None:
            operands.append(bass2jax.partition_id_tensor())
        return tuple(bass2jax._bass_exec_p.bind(
            *operands,
            out_avals=tuple(out_avals),
            in_names=tuple(all_names),
            out_names=tuple(out_names),
            lowering_input_output_aliases=(),
            sim_require_finite=True,
            sim_require_nnan=True,
            nc=nc,
        ))

    devices = jax.devices()[:N_CORES]
    mesh = Mesh(np.asarray(devices), ("core",))
    in_specs = (PartitionSpec("core"),) * (n_params + len(out_names))
    out_specs = (PartitionSpec("core"),) * len(out_names)
    fn = jax.jit(
        shard_map(_body, mesh=mesh, in_specs=in_specs, out_specs=out_specs,
                  check_rep=False),
        donate_argnums=donate, keep_unused=True)

    return {
        "fn": fn, "mesh": mesh, "in_names": in_names,
        "out_names": out_names, "zero_outs": zero_outs,
        "out_avals": out_avals,
    }


def _exec(in_maps, timeit=0):
    """Run the compiled kernel on 8 cores. Returns (results, best_ns).
    timeit>0: additionally re-run the executable with device-staged inputs
    `timeit` times and report the best wall-clock ns."""
    import jax, time
    from jax.sharding import NamedSharding, PartitionSpec

    if "exec" not in _COMPILED:
        _COMPILED["exec"] = _build_exec(_COMPILED["nc"])
    ex = _COMPILED["exec"]
    fn, mesh = ex["fn"], ex["mesh"]
    concat_in = [
        np.concatenate([np.asarray(m[name]) for m in in_maps], axis=0)
        for name in ex["in_names"]
    ]
    concat_zeros = [
        np.zeros((N_CORES * z.shape[0], *z.shape[1:]), z.dtype)
        for z in ex["zero_outs"]
    ]
    out_arrs = fn(*concat_in, *concat_zeros)
    jax.block_until_ready(out_arrs)
    results = [
        {name: np.asarray(out_arrs[i]).reshape(
            N_CORES, *ex["out_avals"][i].shape)[c]
         for i, name in enumerate(ex["out_names"])}
        for c in range(N_CORES)
    ]
    best_ns = None
    if timeit:
        shard = NamedSharding(mesh, PartitionSpec("core"))
        staged = [jax.device_put(x, shard) for x in concat_in]
        jax.block_until_ready(staged)
        for _ in range(timeit):
            zo = [jax.device_put(z, shard) for z in concat_zeros]
            jax.block_until_ready(zo)
            t0 = time.perf_counter()
            o = fn(*staged, *zo)
            jax.block_until_ready(o)
            dt = time.perf_counter() - t0
            best_ns = dt * 1e9 if best_ns is None else min(best_ns, dt * 1e9)
    return results, best_ns


def hw_exec_ns(in_maps, n_long=16, repeats=3):
    """Per-NEFF silicon execution time, measured as the marginal wall-clock
    per execution when n_long executions are dispatched asynchronously
    (PJRT serializes them on the device stream):
        t_neff = (wall(n_long) - wall(1)) / (n_long - 1)
    This removes the constant axon/PJRT dispatch overhead (~68 ms) that a
    single blocking call would include. Returns (t_neff_ns, wall1, wallN)."""
    import jax, time
    from jax.sharding import NamedSharding, PartitionSpec

    if "exec" not in _COMPILED:
        _COMPILED["exec"] = _build_exec(_COMPILED["nc"])
    ex = _COMPILED["exec"]
    fn = ex["fn"]
    shard = NamedSharding(ex["mesh"], PartitionSpec("core"))
    staged = [
        jax.device_put(
            np.concatenate([np.asarray(m[name]) for m in in_maps], axis=0),
            shard)
        for name in ex["in_names"]
    ]
    jax.block_until_ready(staged)
    cz = [np.zeros((N_CORES * z.shape[0], *z.shape[1:]), z.dtype)
          for z in ex["zero_outs"]]

    def run_n(n):
        zos = [[jax.device_put(z, shard) for z in cz] for _ in range(n)]
        for zo in zos:
            jax.block_until_ready(zo)
        t0 = time.perf_counter()
        outs = [fn(*staged, *zo) for zo in zos]
        jax.block_until_ready(outs)
        return time.perf_counter() - t0

    run_n(1)
    run_n(2)  # warm-up
    t1 = min(run_n(1) for _ in range(repeats))
    tn = min(run_n(n_long) for _ in range(repeats))
    return (tn - t1) / (n_long - 1) * 1e9, t1 * 1e9, tn * 1e9


def kernel(**inputs):
    query = np.asarray(inputs["query"], np.float32)
    value = np.asarray(inputs["value"], np.float32)
    reference_points = np.asarray(inputs["reference_points"], np.float32)
    W_off = np.asarray(inputs["W_off"], np.float32)
    b_off = np.asarray(inputs["b_off"], np.float32)
    W_attn = np.asarray(inputs["W_attn"], np.float32)
    b_attn = np.asarray(inputs["b_attn"], np.float32)
    W_val = np.asarray(inputs["W_val"], np.float32)
    b_val = np.asarray(inputs["b_val"], np.float32)
    W_out = np.asarray(inputs["W_out"], np.float32)
    b_out = np.asarray(inputs["b_out"], np.float32)

    if "nc" not in _COMPILED:
        _COMPILED["nc"] = _build_nc()
    nc = _COMPILED["nc"]

    in_maps = _make_in_maps(query, value, reference_points, W_off, b_off,
                            W_attn, b_attn, W_val, b_val, W_out, b_out)
    results, _ = _exec(in_maps)
    outs = [results[b]["out"][:NQ] for b in range(N_CORES)]
    return np.stack(outs, axis=1).astype(np.float32)
